# revision 7
# baseline (speedup 1.0000x reference)
"""ChebyKANLinear Trainium2 kernel.

Math: y[b,o] = (1/I) * sum_{i,d} T_d(c[b,i]) * W[i,o,d],  c = tanh(x)
with Chebyshev T_0=1, T_1=c, T_2=2c^2-1, T_3=4c^3-3c.
(The reference also clips c before arccos; the monomial recombination below
is exact on all of [-1,1], so the clip is irrelevant and dropped.)

Re-expressed in the monomial basis (exact linear recombination, folded into
the weights on the host):
    y = bias + c @ V1 + c^2 @ V2 + c^3 @ V3
    V1 = (W1 - 3*W3)/I, V2 = 2*W2/I, V3 = 4*W3/I, bias_o = sum_i (W0 - W2)[i,o]/I

Sharding: 2D — batch into 4 shards x output_dim into 2 shards across the 8
NeuronCores. Per core the matmuls are computed TRANSPOSED,
    yT[o, b] = sum_k  V_k[i, o].T @ (c^k)[i, b]
so each core runs only 6 matmuls of [K=128, M=128, N=512], and the bias
becomes a per-partition scalar fused into the PSUM->SBUF merge.

Everything rides in bf16 (rel-err budget is 2e-2; measured bf16 error is
~7e-3): halves the input DMA bytes, and a bf16 matmul is ONE PE pass where
fp32 needs two (LOW_HIGH split with doubled LDWEIGHTS). PSUM stays fp32.

Perf notes from v1-v3 trace analysis:
- Per-DMA end-to-end ~2.4us fixed (DGE pipe + 0.9us completion-semaphore
  propagation) and concurrent descriptor streams starve each other's
  completion tails (v3: 4 concurrent input DMAs pushed the FIRST
  completion from 9.4us to 10.9us). So exactly TWO input DMAs: x (both
  i-halves packed [128,1024] on the sync queue, lands ~10.0us) and V+bias
  ([128,769] on the scalar queue, lands ~11.2us).
- Warmup matmuls (fp32 on memset tiles, two passes each) keep the PE HAM
  clock-gate (1.2 -> 2.4 GHz) open until the real chain starts; a PE idle
  gap resets it (v3: 1.4us gap made every real matmul 630ns instead of
  375ns).
- Matmul order alternates PSUM banks in operand-arrival order; the last
  matmul is split into two N=256 passes so the first y-half merge + store
  can start one pass early.
- Tail: acc_a+bias pre-merge split ACT(Identity+bias)/DVE, final
  y = acc_b + tmp on DVE, two half out-DMAs on the two HWDGE queues.
"""

from contextlib import ExitStack

import numpy as np
import ml_dtypes

import concourse.bass as bass
import concourse.tile as tile
from concourse import bacc, mybir
from concourse.bass_utils import run_bass_kernel_spmd

N_CORES = 8
B, I, O, D = 2048, 256, 256, 4
RB, SO = 4, 2  # batch shards x output shards
BL = B // RB  # 512 batch rows per core
OL = O // SO  # 128 output cols per core
F32 = mybir.dt.float32
BF16 = mybir.dt.bfloat16
NP_BF16 = ml_dtypes.bfloat16

# packed weight-column offsets (matmul need-order), bias last
_COL = {
    (0, 0): 0,
    (1, 0): OL,
    (0, 1): 2 * OL,
    (2, 0): 3 * OL,
    (1, 1): 4 * OL,
    (2, 1): 5 * OL,
    "bias": 6 * OL,
}
VB_W = 6 * OL + 1

_cache = {}


def _build_program():
    nc = bacc.Bacc("TRN2", target_bir_lowering=False, debug=False, num_devices=N_CORES)

    # x pre-transposed + bf16-cast on host; both i-halves side by side:
    # cols [0,512) = i 0:128, cols [512,1024) = i 128:256
    xt_d = nc.dram_tensor("xt", [128, 2 * BL], BF16, kind="ExternalInput")
    vb_d = nc.dram_tensor("vb", [128, VB_W], BF16, kind="ExternalInput")
    # transposed output [o_local, b_local], bf16 (host casts back to fp32)
    y_d = nc.dram_tensor("y", [OL, BL], BF16, kind="ExternalOutput")

    with tile.TileContext(nc) as tc, ExitStack() as ctx:
        pool = ctx.enter_context(tc.tile_pool(name="main", bufs=1))
        psum = ctx.enter_context(
            tc.tile_pool(name="psum", bufs=1, space=bass.MemorySpace.PSUM)
        )

        # PE warmup operands (DVE is idle this early; values are irrelevant)
        wu_w = pool.tile([128, 128], F32, tag="wu_w")
        nc.vector.memset(wu_w[:], 1.0)
        wu_r = pool.tile([128, 512], F32, tag="wu_r")
        nc.vector.memset(wu_r[:], 1.0)

        vb = pool.tile([128, VB_W], BF16, tag="vb")
        xt = pool.tile([128, 2 * BL], BF16, tag="xt")
        nc.sync.dma_start(xt[:], xt_d[:])
        nc.scalar.dma_start(vb[:], vb_d[:])
        xh = {0: xt[:, :BL], 1: xt[:, BL:]}

        # Warmup: fp32 (two LOW/HIGH passes each — maximum PE busy-time per
        # instruction), sized to end right as V lands (~11.2us).
        wu_acc = psum.tile([128, 512], F32, tag="wu_acc")
        nc.tensor.matmul(wu_acc[:], wu_w[:], wu_r[:], start=True, stop=True)
        nc.tensor.matmul(
            wu_acc[:, :256], wu_w[:], wu_r[:, :256], start=True, stop=True
        )
        nc.tensor.matmul(
            wu_acc[:, :128], wu_w[:], wu_r[:, :128], start=True, stop=True
        )

        # basis: c = tanh(xT) on ACT (h1 split so DVE can start its squares
        # sooner), c^2/c^3 on DVE (all bf16)
        basis = {}
        hb = BL // 2
        c0 = pool.tile([128, BL], BF16, tag="c0")
        nc.scalar.activation(c0[:], xh[0], mybir.ActivationFunctionType.Tanh)
        basis[(0, 0)] = c0
        c1 = pool.tile([128, BL], BF16, tag="c1")
        nc.scalar.activation(c1[:, :hb], xh[1][:, :hb], mybir.ActivationFunctionType.Tanh)
        nc.scalar.activation(c1[:, hb:], xh[1][:, hb:], mybir.ActivationFunctionType.Tanh)
        basis[(0, 1)] = c1
        c2_0 = pool.tile([128, BL], BF16, tag="c2_0")
        nc.vector.tensor_mul(c2_0[:], c0[:], c0[:])
        basis[(1, 0)] = c2_0
        c3_0 = pool.tile([128, BL], BF16, tag="c3_0")
        nc.vector.tensor_mul(c3_0[:], c2_0[:], c0[:])
        basis[(2, 0)] = c3_0
        c2_1 = pool.tile([128, BL], BF16, tag="c2_1")
        c3_1 = pool.tile([128, BL], BF16, tag="c3_1")
        for k in range(2):
            s = slice(k * hb, (k + 1) * hb)
            nc.vector.tensor_mul(c2_1[:, s], c1[:, s], c1[:, s])
            nc.vector.tensor_mul(c3_1[:, s], c2_1[:, s], c1[:, s])
        basis[(1, 1)] = c2_1
        basis[(2, 1)] = c3_1

        # tensor_scalar/activation need an fp32 scalar: GpSimd (idle all
        # kernel, cannot touch PSUM) up-converts the bf16 bias column.
        bias_col = pool.tile([128, 1], F32, tag="bias_f32")
        nc.gpsimd.tensor_copy(bias_col[:], vb[:, _COL["bias"] : _COL["bias"] + 1])

        # yT[o, b]: 6 matmuls alternating PSUM banks (ih=0 -> acc_a,
        # ih=1 -> acc_b) in operand-arrival order; (2,1) split N=256 so the
        # first y-half merge can start one pass early.
        acc_a = psum.tile([128, BL], F32, tag="acc_a")
        acc_b = psum.tile([128, BL], F32, tag="acc_b")
        accs = {0: acc_a, 1: acc_b}
        for d, ih in [(0, 0), (1, 0), (0, 1), (2, 0), (1, 1)]:
            col = _COL[(d, ih)]
            nc.tensor.matmul(
                accs[ih][:OL, :],
                vb[:, col : col + OL],
                basis[(d, ih)][:],
                start=(d == 0),
                stop=(d == 2),
            )
        col = _COL[(2, 1)]
        nc.tensor.matmul(
            acc_b[:OL, :hb], vb[:, col : col + OL], c3_1[:, :hb],
            start=False, stop=True,
        )
        nc.tensor.matmul(
            acc_b[:OL, hb:], vb[:, col : col + OL], c3_1[:, hb:],
            start=False, stop=True,
        )

        # DVE can read only ONE PSUM operand per op: pre-merge acc_a + bias
        # into SBUF while acc_b's matmuls still run (ACT does one half via
        # Identity+bias — it can read PSUM and is idle after the tanhs —
        # DVE the other), then y = acc_b + tmp per half, store immediately.
        tmp_sb = pool.tile([OL, BL], BF16, tag="tmp_sb")
        y_sb = pool.tile([OL, BL], BF16, tag="y_sb")
        nc.scalar.activation(
            tmp_sb[:, :hb],
            acc_a[:OL, :hb],
            mybir.ActivationFunctionType.Identity,
            bias=bias_col[:],
        )
        nc.vector.tensor_scalar_add(tmp_sb[:, hb:], acc_a[:OL, hb:], bias_col[:])
        for k in range(2):
            s = slice(k * hb, (k + 1) * hb)
            nc.vector.tensor_tensor(
                y_sb[:, s], acc_b[:OL, s], tmp_sb[:, s], mybir.AluOpType.add
            )
            (nc.sync if k == 0 else nc.scalar).dma_start(y_d[:, s], y_sb[:, s])

    nc.compile()
    return nc


def _get_program():
    if "nc" not in _cache:
        _cache["nc"] = _build_program()
    return _cache["nc"]


def _make_in_maps(x, cheby_coeffs):
    x = np.ascontiguousarray(x, dtype=np.float32)
    W = np.ascontiguousarray(cheby_coeffs, dtype=np.float32)
    assert x.shape == (B, I) and W.shape == (I, O, D)

    inv_i = np.float32(1.0 / I)
    V = np.stack(
        [
            W[:, :, 1] - 3.0 * W[:, :, 3],
            2.0 * W[:, :, 2],
            4.0 * W[:, :, 3],
        ]
    ).astype(np.float32) * inv_i  # [3, I, O]
    bias_full = (W[:, :, 0] - W[:, :, 2]).sum(axis=0, dtype=np.float32) * inv_i  # [O]

    xt_shards = []
    for rb in range(RB):
        xs = x[rb * BL : (rb + 1) * BL, :].T.astype(NP_BF16)  # [I, BL]
        # [128, 2*BL]: cols [0,BL) = i 0:128, cols [BL,2BL) = i 128:256
        packed = np.concatenate([xs[:128, :], xs[128:, :]], axis=1)
        xt_shards.append(np.ascontiguousarray(packed))
    vb_shards = []
    for so in range(SO):
        vb = np.empty((128, VB_W), dtype=NP_BF16)
        osl = slice(so * OL, (so + 1) * OL)
        for key, col in _COL.items():
            if key == "bias":
                continue
            d, ih = key
            vb[:, col : col + OL] = V[d, ih * 128 : (ih + 1) * 128, osl].astype(
                NP_BF16
            )
        vb[:, _COL["bias"]] = bias_full[osl].astype(NP_BF16)
        vb_shards.append(vb)
    in_maps = []
    for c_id in range(N_CORES):
        rb, so = divmod(c_id, SO)
        in_maps.append({"xt": xt_shards[rb], "vb": vb_shards[so]})
    return in_maps


def kernel(x, cheby_coeffs):
    nc = _get_program()
    in_maps = _make_in_maps(x, cheby_coeffs)
    res = run_bass_kernel_spmd(nc, in_maps, list(range(N_CORES)))
    y = np.empty((B, O), dtype=np.float32)
    for c_id in range(N_CORES):
        rb, so = divmod(c_id, SO)
        y[rb * BL : (rb + 1) * BL, so * OL : (so + 1) * OL] = (
            res.results[c_id]["y"].astype(np.float32).T
        )
    return y


# revision 9
# speedup vs baseline: 1.0591x; 1.0591x over previous
"""ChebyKANLinear Trainium2 kernel.

Math: y[b,o] = (1/I) * sum_{i,d} T_d(c[b,i]) * W[i,o,d],  c = tanh(x)
with Chebyshev T_0=1, T_1=c, T_2=2c^2-1, T_3=4c^3-3c.
(The reference also clips c before arccos; the monomial recombination below
is exact on all of [-1,1], so the clip is irrelevant and dropped.)

Re-expressed in the monomial basis (exact linear recombination, folded into
the weights on the host):
    y = bias + c @ V1 + c^2 @ V2 + c^3 @ V3
    V1 = (W1 - 3*W3)/I, V2 = 2*W2/I, V3 = 4*W3/I, bias_o = sum_i (W0 - W2)[i,o]/I

Sharding: 2D — batch into 4 shards x output_dim into 2 shards across the 8
NeuronCores. Per core the matmuls are computed TRANSPOSED,
    yT[o, b] = sum_k  V_k[i, o].T @ (c^k)[i, b]
so each core runs only 6 matmuls of [K=128, M=128, N=512], and the bias
becomes a per-partition scalar fused into the PSUM->SBUF merge.

Everything rides in bf16 (rel-err budget is 2e-2; measured bf16 error is
~7e-3): halves the input DMA bytes, and a bf16 matmul is ONE PE pass where
fp32 needs two (LOW_HIGH split with doubled LDWEIGHTS). PSUM stays fp32.

Perf notes from v1-v4 trace analysis:
- Per-DMA end-to-end ~2.4us fixed (DGE pipe + 0.9us completion-semaphore
  propagation). Worse: when BOTH HWDGE queues stream concurrently, each
  DMA's final descriptors (one lagging DMA engine's share) are starved
  until every concurrent stream drains (v4: x's data was in SBUF at 9.8us
  but its completion fired at 11.6us). So both input DMAs ride ONE queue
  (sync), serialized: x (both i-halves packed [128,1024]) first, V+bias
  ([128,769]) second — nothing else streams, completions fire promptly.
- Warmup matmuls (fp32 on memset tiles, two passes each) keep the PE HAM
  clock-gate (1.2 -> 2.4 GHz) open until the real chain starts; a PE idle
  gap resets it (v3/v4: a ~1us gap made every real matmul 630ns instead
  of 375ns). Warmup operands are memset on GpSimd right after the
  framework consts so the warmup chain starts ~7.0us and spans to the
  real chain.
- Matmul order alternates PSUM banks in operand-arrival order; the last
  matmul is split into two N=256 passes so the first y-half merge + store
  can start one pass early.
- Tail: acc_a+bias pre-merge split ACT(Identity+bias)/DVE, final
  y = acc_b + tmp on DVE, two half out-DMAs on the two HWDGE queues.
"""

from contextlib import ExitStack

import numpy as np
import ml_dtypes

import concourse.bass as bass
import concourse.tile as tile
from concourse import bacc, mybir
from concourse.bass_utils import run_bass_kernel_spmd

N_CORES = 8
B, I, O, D = 2048, 256, 256, 4
RB, SO = 4, 2  # batch shards x output shards
BL = B // RB  # 512 batch rows per core
OL = O // SO  # 128 output cols per core
F32 = mybir.dt.float32
BF16 = mybir.dt.bfloat16
NP_BF16 = ml_dtypes.bfloat16

# packed weight-column offsets (matmul need-order), bias last
_COL = {
    (0, 0): 0,
    (1, 0): OL,
    (0, 1): 2 * OL,
    (2, 0): 3 * OL,
    (1, 1): 4 * OL,
    (2, 1): 5 * OL,
    "bias": 6 * OL,
}
VB_W = 6 * OL + 1

_cache = {}


def _build_program():
    nc = bacc.Bacc("TRN2", target_bir_lowering=False, debug=False, num_devices=N_CORES)

    # x pre-transposed + bf16-cast on host; both i-halves side by side:
    # cols [0,512) = i 0:128, cols [512,1024) = i 128:256
    xt_d = nc.dram_tensor("xt", [128, 2 * BL], BF16, kind="ExternalInput")
    vb_d = nc.dram_tensor("vb", [128, VB_W], BF16, kind="ExternalInput")
    # transposed output [o_local, b_local], bf16 (host casts back to fp32)
    y_d = nc.dram_tensor("y", [OL, BL], BF16, kind="ExternalOutput")

    with tile.TileContext(nc) as tc, ExitStack() as ctx:
        pool = ctx.enter_context(tc.tile_pool(name="main", bufs=1))
        psum = ctx.enter_context(
            tc.tile_pool(name="psum", bufs=1, space=bass.MemorySpace.PSUM)
        )

        # PE warmup operands: memset on GpSimd (free right after the
        # framework const memsets ~6.2us) so the warmup chain starts early
        wu_w = pool.tile([128, 128], F32, tag="wu_w")
        nc.gpsimd.memset(wu_w[:], 1.0)
        wu_r = pool.tile([128, 512], F32, tag="wu_r")
        nc.gpsimd.memset(wu_r[:], 1.0)

        vb = pool.tile([128, VB_W], BF16, tag="vb")
        xt = pool.tile([128, 2 * BL], BF16, tag="xt")
        nc.sync.dma_start(xt[:], xt_d[:])
        nc.sync.dma_start(vb[:], vb_d[:])
        xh = {0: xt[:, :BL], 1: xt[:, BL:]}

        # Warmup: fp32 (two LOW/HIGH passes each — maximum PE busy-time per
        # instruction), spanning ~7.0us to the real chain start (~10.9us).
        wu_acc = psum.tile([128, 512], F32, tag="wu_acc")
        nc.tensor.matmul(wu_acc[:], wu_w[:], wu_r[:], start=True, stop=True)
        nc.tensor.matmul(
            wu_acc[:, :256], wu_w[:], wu_r[:, :256], start=True, stop=True
        )
        nc.tensor.matmul(
            wu_acc[:, :128], wu_w[:], wu_r[:, :128], start=True, stop=True
        )

        # basis: c = tanh(xT) on ACT (h1 split so DVE can start its squares
        # sooner), c^2/c^3 on DVE (all bf16)
        basis = {}
        hb = BL // 2
        c0 = pool.tile([128, BL], BF16, tag="c0")
        nc.scalar.activation(c0[:], xh[0], mybir.ActivationFunctionType.Tanh)
        basis[(0, 0)] = c0
        c1 = pool.tile([128, BL], BF16, tag="c1")
        nc.scalar.activation(c1[:, :hb], xh[1][:, :hb], mybir.ActivationFunctionType.Tanh)
        nc.scalar.activation(c1[:, hb:], xh[1][:, hb:], mybir.ActivationFunctionType.Tanh)
        basis[(0, 1)] = c1
        c2_0 = pool.tile([128, BL], BF16, tag="c2_0")
        nc.vector.tensor_mul(c2_0[:], c0[:], c0[:])
        basis[(1, 0)] = c2_0
        c3_0 = pool.tile([128, BL], BF16, tag="c3_0")
        nc.vector.tensor_mul(c3_0[:], c2_0[:], c0[:])
        basis[(2, 0)] = c3_0
        c2_1 = pool.tile([128, BL], BF16, tag="c2_1")
        c3_1 = pool.tile([128, BL], BF16, tag="c3_1")
        for k in range(2):
            s = slice(k * hb, (k + 1) * hb)
            nc.vector.tensor_mul(c2_1[:, s], c1[:, s], c1[:, s])
            nc.vector.tensor_mul(c3_1[:, s], c2_1[:, s], c1[:, s])
        basis[(1, 1)] = c2_1
        basis[(2, 1)] = c3_1

        # tensor_scalar/activation need an fp32 scalar: GpSimd (idle all
        # kernel, cannot touch PSUM) up-converts the bf16 bias column.
        bias_col = pool.tile([128, 1], F32, tag="bias_f32")
        nc.gpsimd.tensor_copy(bias_col[:], vb[:, _COL["bias"] : _COL["bias"] + 1])

        # yT[o, b]: 6 matmuls alternating PSUM banks (ih=0 -> acc_a,
        # ih=1 -> acc_b) in operand-arrival order; (2,1) split N=256 so the
        # first y-half merge can start one pass early.
        acc_a = psum.tile([128, BL], F32, tag="acc_a")
        acc_b = psum.tile([128, BL], F32, tag="acc_b")
        accs = {0: acc_a, 1: acc_b}
        for d, ih in [(0, 0), (1, 0), (0, 1), (2, 0), (1, 1)]:
            col = _COL[(d, ih)]
            nc.tensor.matmul(
                accs[ih][:OL, :],
                vb[:, col : col + OL],
                basis[(d, ih)][:],
                start=(d == 0),
                stop=(d == 2),
            )
        col = _COL[(2, 1)]
        nc.tensor.matmul(
            acc_b[:OL, :hb], vb[:, col : col + OL], c3_1[:, :hb],
            start=False, stop=True,
        )
        nc.tensor.matmul(
            acc_b[:OL, hb:], vb[:, col : col + OL], c3_1[:, hb:],
            start=False, stop=True,
        )

        # DVE can read only ONE PSUM operand per op: pre-merge acc_a + bias
        # into SBUF while acc_b's matmuls still run (ACT does one half via
        # Identity+bias — it can read PSUM and is idle after the tanhs —
        # DVE the other), then y = acc_b + tmp per half, store immediately.
        tmp_sb = pool.tile([OL, BL], BF16, tag="tmp_sb")
        y_sb = pool.tile([OL, BL], BF16, tag="y_sb")
        nc.scalar.activation(
            tmp_sb[:, :hb],
            acc_a[:OL, :hb],
            mybir.ActivationFunctionType.Identity,
            bias=bias_col[:],
        )
        nc.vector.tensor_scalar_add(tmp_sb[:, hb:], acc_a[:OL, hb:], bias_col[:])
        for k in range(2):
            s = slice(k * hb, (k + 1) * hb)
            nc.vector.tensor_tensor(
                y_sb[:, s], acc_b[:OL, s], tmp_sb[:, s], mybir.AluOpType.add
            )
            (nc.sync if k == 0 else nc.scalar).dma_start(y_d[:, s], y_sb[:, s])

    nc.compile()
    return nc


def _get_program():
    if "nc" not in _cache:
        _cache["nc"] = _build_program()
    return _cache["nc"]


def _make_in_maps(x, cheby_coeffs):
    x = np.ascontiguousarray(x, dtype=np.float32)
    W = np.ascontiguousarray(cheby_coeffs, dtype=np.float32)
    assert x.shape == (B, I) and W.shape == (I, O, D)

    inv_i = np.float32(1.0 / I)
    V = np.stack(
        [
            W[:, :, 1] - 3.0 * W[:, :, 3],
            2.0 * W[:, :, 2],
            4.0 * W[:, :, 3],
        ]
    ).astype(np.float32) * inv_i  # [3, I, O]
    bias_full = (W[:, :, 0] - W[:, :, 2]).sum(axis=0, dtype=np.float32) * inv_i  # [O]

    xt_shards = []
    for rb in range(RB):
        xs = x[rb * BL : (rb + 1) * BL, :].T.astype(NP_BF16)  # [I, BL]
        # [128, 2*BL]: cols [0,BL) = i 0:128, cols [BL,2BL) = i 128:256
        packed = np.concatenate([xs[:128, :], xs[128:, :]], axis=1)
        xt_shards.append(np.ascontiguousarray(packed))
    vb_shards = []
    for so in range(SO):
        vb = np.empty((128, VB_W), dtype=NP_BF16)
        osl = slice(so * OL, (so + 1) * OL)
        for key, col in _COL.items():
            if key == "bias":
                continue
            d, ih = key
            vb[:, col : col + OL] = V[d, ih * 128 : (ih + 1) * 128, osl].astype(
                NP_BF16
            )
        vb[:, _COL["bias"]] = bias_full[osl].astype(NP_BF16)
        vb_shards.append(vb)
    in_maps = []
    for c_id in range(N_CORES):
        rb, so = divmod(c_id, SO)
        in_maps.append({"xt": xt_shards[rb], "vb": vb_shards[so]})
    return in_maps


def kernel(x, cheby_coeffs):
    nc = _get_program()
    in_maps = _make_in_maps(x, cheby_coeffs)
    res = run_bass_kernel_spmd(nc, in_maps, list(range(N_CORES)))
    y = np.empty((B, O), dtype=np.float32)
    for c_id in range(N_CORES):
        rb, so = divmod(c_id, SO)
        y[rb * BL : (rb + 1) * BL, so * OL : (so + 1) * OL] = (
            res.results[c_id]["y"].astype(np.float32).T
        )
    return y


# revision 16
# speedup vs baseline: 1.1139x; 1.0517x over previous
"""ChebyKANLinear Trainium2 kernel.

Math: y[b,o] = (1/I) * sum_{i,d} T_d(c[b,i]) * W[i,o,d],  c = tanh(x)
with Chebyshev T_0=1, T_1=c, T_2=2c^2-1, T_3=4c^3-3c.
(The reference also clips c before arccos; the monomial recombination below
is exact on all of [-1,1], so the clip is irrelevant and dropped.)

Re-expressed in the monomial basis (exact linear recombination, folded into
the weights on the host):
    y = bias + c @ V1 + c^2 @ V2 + c^3 @ V3
    V1 = (W1 - 3*W3)/I, V2 = 2*W2/I, V3 = 4*W3/I, bias_o = sum_i (W0 - W2)[i,o]/I

Sharding: 2D — batch into 4 shards x output_dim into 2 shards across the 8
NeuronCores. Per core the matmuls are computed TRANSPOSED,
    yT[o, b] = sum_k  V_k[i, o].T @ (c^k)[i, b]
so each core runs only 6 matmuls of [K=128, M=128, N=512], and the bias
becomes a per-partition scalar fused into the PSUM->SBUF merge.

Everything rides in bf16 (rel-err budget is 2e-2; measured bf16 error is
~7e-3): halves the input DMA bytes, and a bf16 matmul is ONE PE pass where
fp32 needs two (LOW_HIGH split with doubled LDWEIGHTS). PSUM stays fp32.

Perf notes from v1-v4 trace analysis:
- Per-DMA end-to-end ~2.4us fixed (DGE pipe + 0.9us completion-semaphore
  propagation). Worse: when BOTH HWDGE queues stream concurrently, each
  DMA's final descriptors (one lagging DMA engine's share) are starved
  until every concurrent stream drains (v4: x's data was in SBUF at 9.8us
  but its completion fired at 11.6us). So both input DMAs ride ONE queue
  (sync), serialized: x (both i-halves packed [128,1024]) first, V+bias
  ([128,769]) second — nothing else streams, completions fire promptly.
- Warmup matmuls (fp32 on memset tiles, two passes each) keep the PE HAM
  clock-gate (1.2 -> 2.4 GHz) open until the real chain starts; a PE idle
  gap resets it (v3/v4: a ~1us gap made every real matmul 630ns instead
  of 375ns). Warmup operands are memset on GpSimd right after the
  framework consts so the warmup chain starts ~7.0us and spans to the
  real chain.
- Matmul order alternates PSUM banks in operand-arrival order; the last
  matmul is split into two N=256 passes so the first y-half merge + store
  can start one pass early.
- Tail: acc_a+bias pre-merge split ACT(Identity+bias)/DVE, final
  y = acc_b + tmp on DVE, two half out-DMAs on the two HWDGE queues.
"""

from contextlib import ExitStack

import numpy as np
import ml_dtypes

import concourse.bass as bass
import concourse.tile as tile
from concourse import bacc, mybir
from concourse.bass_utils import run_bass_kernel_spmd

N_CORES = 8
B, I, O, D = 2048, 256, 256, 4
RB, SO = 4, 2  # batch shards x output shards
BL = B // RB  # 512 batch rows per core
OL = O // SO  # 128 output cols per core
F32 = mybir.dt.float32
BF16 = mybir.dt.bfloat16
NP_BF16 = ml_dtypes.bfloat16

# packed weight-column offsets (matmul need-order), bias last
_COL = {
    (0, 0): 0,
    (1, 0): OL,
    (0, 1): 2 * OL,
    (2, 0): 3 * OL,
    (1, 1): 4 * OL,
    (2, 1): 5 * OL,
    "bias": 6 * OL,
}
VB_W = 6 * OL + 1

_cache = {}


def _build_program():
    nc = bacc.Bacc("TRN2", target_bir_lowering=False, debug=False, num_devices=N_CORES)

    # DMA chunk 1: x i-half 0 (pre-transposed, bf16) packed side by side
    # with ALL weights + bias — one completion unlocks tanh(h0) AND the
    # matmul chain. DMA chunk 2: x i-half 1 (its consumers run later).
    xv_d = nc.dram_tensor("xv", [128, BL + VB_W], BF16, kind="ExternalInput")
    x1_d = nc.dram_tensor("x1", [128, BL], BF16, kind="ExternalInput")
    # transposed output [o_local, b_local], bf16 (host casts back to fp32)
    y_d = nc.dram_tensor("y", [OL, BL], BF16, kind="ExternalOutput")

    with tile.TileContext(nc) as tc, ExitStack() as ctx:
        pool = ctx.enter_context(tc.tile_pool(name="main", bufs=1))
        psum = ctx.enter_context(
            tc.tile_pool(name="psum", bufs=1, space=bass.MemorySpace.PSUM)
        )

        # PE warmup operands: memset on GpSimd (free right after the
        # framework const memsets ~6.2us) so the warmup chain starts early
        wu_w = pool.tile([128, 128], F32, tag="wu_w")
        nc.gpsimd.memset(wu_w[:], 1.0)
        wu_r = pool.tile([128, 512], F32, tag="wu_r")
        nc.gpsimd.memset(wu_r[:], 1.0)

        xv = pool.tile([128, BL + VB_W], BF16, tag="xv")
        x1 = pool.tile([128, BL], BF16, tag="x1")
        nc.sync.dma_start(xv[:], xv_d[:])
        nc.sync.dma_start(x1[:], x1_d[:])
        xh = {0: xv[:, :BL], 1: x1[:]}

        def vcol(col, width=OL):
            return xv[:, BL + col : BL + col + width]

        # Warmup: fp32 (two LOW/HIGH passes each — maximum PE busy-time per
        # instruction), spanning ~7.0us to the real chain start (~10.6us);
        # the trailing small ones bridge the gap so the clock never drops.
        wu_acc = psum.tile([128, 512], F32, tag="wu_acc")
        nc.tensor.matmul(wu_acc[:], wu_w[:], wu_r[:], start=True, stop=True)
        nc.tensor.matmul(
            wu_acc[:, :256], wu_w[:], wu_r[:, :256], start=True, stop=True
        )
        nc.tensor.matmul(
            wu_acc[:, :128], wu_w[:], wu_r[:, :128], start=True, stop=True
        )
        nc.tensor.matmul(
            wu_acc[:, :64], wu_w[:], wu_r[:, :64], start=True, stop=True
        )

        # basis: c = tanh(xT) on ACT (h1 split so DVE can start its squares
        # sooner), c^2/c^3 on DVE (all bf16)
        basis = {}
        hb = BL // 2
        c0 = pool.tile([128, BL], BF16, tag="c0")
        nc.scalar.activation(c0[:], xh[0], mybir.ActivationFunctionType.Tanh)
        basis[(0, 0)] = c0
        c1 = pool.tile([128, BL], BF16, tag="c1")
        nc.scalar.activation(c1[:, :hb], xh[1][:, :hb], mybir.ActivationFunctionType.Tanh)
        nc.scalar.activation(c1[:, hb:], xh[1][:, hb:], mybir.ActivationFunctionType.Tanh)
        basis[(0, 1)] = c1
        c2_0 = pool.tile([128, BL], BF16, tag="c2_0")
        nc.vector.tensor_mul(c2_0[:], c0[:], c0[:])
        basis[(1, 0)] = c2_0
        c3_0 = pool.tile([128, BL], BF16, tag="c3_0")
        nc.vector.tensor_mul(c3_0[:], c2_0[:], c0[:])
        basis[(2, 0)] = c3_0
        c2_1 = pool.tile([128, BL], BF16, tag="c2_1")
        c3_1 = pool.tile([128, BL], BF16, tag="c3_1")
        for k in range(2):
            s = slice(k * hb, (k + 1) * hb)
            nc.vector.tensor_mul(c2_1[:, s], c1[:, s], c1[:, s])
            nc.vector.tensor_mul(c3_1[:, s], c2_1[:, s], c1[:, s])
        basis[(1, 1)] = c2_1
        basis[(2, 1)] = c3_1

        # activation bias needs an fp32 scalar: GpSimd (idle all kernel,
        # cannot touch PSUM) up-converts the bf16 bias column.
        bias_col = pool.tile([128, 1], F32, tag="bias_f32")
        nc.gpsimd.tensor_copy(bias_col[:], vcol(_COL["bias"], 1))

        # yT[o, b]: 6 matmuls alternating PSUM banks (ih=0 -> acc_a,
        # ih=1 -> acc_b) in operand-arrival order; (2,1) split N=256 so the
        # first y-half merge can start one pass early.
        acc_a = psum.tile([128, BL], F32, tag="acc_a")
        acc_b = psum.tile([128, BL], F32, tag="acc_b")
        accs = {0: acc_a, 1: acc_b}
        for d, ih in [(0, 0), (1, 0), (0, 1), (2, 0), (1, 1)]:
            nc.tensor.matmul(
                accs[ih][:OL, :],
                vcol(_COL[(d, ih)]),
                basis[(d, ih)][:],
                start=(d == 0),
                stop=(d == 2),
            )
        nc.tensor.matmul(
            acc_b[:OL, :hb], vcol(_COL[(2, 1)]), c3_1[:, :hb],
            start=False, stop=True,
        )
        nc.tensor.matmul(
            acc_b[:OL, hb:], vcol(_COL[(2, 1)]), c3_1[:, hb:],
            start=False, stop=True,
        )

        # DVE can read only ONE PSUM operand per op: pre-merge acc_a + bias
        # into SBUF while acc_b's matmuls still run. BOTH halves on ACT
        # (Identity+bias reads PSUM; ACT is idle after the tanhs) so DVE is
        # free to fire each y = acc_b + tmp the moment (2,1)'s half pass
        # retires; each y half's store issues immediately on its own queue.
        tmp_sb = pool.tile([OL, BL], BF16, tag="tmp_sb")
        y_sb = pool.tile([OL, BL], BF16, tag="y_sb")
        for k in range(2):
            s = slice(k * hb, (k + 1) * hb)
            nc.scalar.activation(
                tmp_sb[:, s],
                acc_a[:OL, s],
                mybir.ActivationFunctionType.Identity,
                bias=bias_col[:],
            )
        for k in range(2):
            s = slice(k * hb, (k + 1) * hb)
            nc.vector.tensor_tensor(
                y_sb[:, s], acc_b[:OL, s], tmp_sb[:, s], mybir.AluOpType.add
            )
            (nc.sync if k == 0 else nc.scalar).dma_start(y_d[:, s], y_sb[:, s])

    nc.compile()
    return nc


def _get_program():
    if "nc" not in _cache:
        _cache["nc"] = _build_program()
    return _cache["nc"]


def _make_in_maps(x, cheby_coeffs):
    x = np.ascontiguousarray(x, dtype=np.float32)
    W = np.ascontiguousarray(cheby_coeffs, dtype=np.float32)
    assert x.shape == (B, I) and W.shape == (I, O, D)

    inv_i = np.float32(1.0 / I)
    V = np.stack(
        [
            W[:, :, 1] - 3.0 * W[:, :, 3],
            2.0 * W[:, :, 2],
            4.0 * W[:, :, 3],
        ]
    ).astype(np.float32) * inv_i  # [3, I, O]
    bias_full = (W[:, :, 0] - W[:, :, 2]).sum(axis=0, dtype=np.float32) * inv_i  # [O]

    x0_shards, x1_shards = [], []
    for rb in range(RB):
        xs = x[rb * BL : (rb + 1) * BL, :].T.astype(NP_BF16)  # [I, BL]
        x0_shards.append(xs[:128, :])
        x1_shards.append(np.ascontiguousarray(xs[128:, :]))
    vb_shards = []
    for so in range(SO):
        vb = np.empty((128, VB_W), dtype=NP_BF16)
        osl = slice(so * OL, (so + 1) * OL)
        for key, col in _COL.items():
            if key == "bias":
                continue
            d, ih = key
            vb[:, col : col + OL] = V[d, ih * 128 : (ih + 1) * 128, osl].astype(
                NP_BF16
            )
        vb[:, _COL["bias"]] = bias_full[osl].astype(NP_BF16)
        vb_shards.append(vb)
    in_maps = []
    for c_id in range(N_CORES):
        rb, so = divmod(c_id, SO)
        xv = np.ascontiguousarray(
            np.concatenate([x0_shards[rb], vb_shards[so]], axis=1)
        )
        in_maps.append({"xv": xv, "x1": x1_shards[rb]})
    return in_maps


def kernel(x, cheby_coeffs):
    nc = _get_program()
    in_maps = _make_in_maps(x, cheby_coeffs)
    res = run_bass_kernel_spmd(nc, in_maps, list(range(N_CORES)))
    y = np.empty((B, O), dtype=np.float32)
    for c_id in range(N_CORES):
        rb, so = divmod(c_id, SO)
        y[rb * BL : (rb + 1) * BL, so * OL : (so + 1) * OL] = (
            res.results[c_id]["y"].astype(np.float32).T
        )
    return y
